# revision 1
# baseline (speedup 1.0000x reference)
"""Trainium2 Bass kernel for the DialogGCN GAT-style message-passing layer.

Math notes (why this is much cheaper than the reference graph):
  Kp    = concat(K, kfeat) @ Wk + bk                    (B,N,D)
  alpha = Q@wden[:D] + Kp@wden[D:] + bden               (B,N)
  w     = softmax(alpha - (1-adj)*1e30, axis=N)
  out   = sum_n w * ((Kp@Wr0)*sm + (Kp@Wr1)*(1-sm))

* softmax is invariant to per-row constants, so the Q term, bden and the
  bk@wden[D:] constant all cancel:  w = softmax_n(X_n . v) masked, where
  X = concat(K, kfeat) and v = Wk @ wden[D:]  (folded on host).
* the output is linear in the weighted sums:
    out = (sum_n w*sm*X_n | c0) @ [Wk;bk] @ Wr0 + (sum_n w*(1-sm)*X_n | c1) @ [Wk;bk] @ Wr1
  so G0 = [Wk;bk]@Wr0 and G1 = [Wk;bk]@Wr1 are folded on host (769x512 each)
  and the device only needs one streaming pass over X computing
    s_n = X_n . v ; p_n = exp(s_n)*adj_n ; U0 = sum p*sm*[X|1] ; U1 = sum p*(1-sm)*[X|1]
  followed by a tiny projection (U0@G0 + U1@G1) / P.

Sharding: pure data parallel over batch B=32 across 8 cores (4 rows each).
"""

import os
import sys

import numpy as np

for _p in ("/opt/trn_rl_repo", "/root/.axon_site/_ro/trn_rl_repo"):
    if os.path.isdir(_p) and _p not in sys.path:
        sys.path.insert(0, _p)

B, N, D, KD = 32, 2048, 512, 256
F = D + KD  # 768
NCORES = 8
BL = B // NCORES  # 4 batch rows per core
NT = 16  # free-dim token tiles per batch (N = 128 * NT)

_BUILD_CACHE = {}
last_results = None  # BassKernelResults of the most recent run (for test.py)


def _build(stream_f32r: bool):
    """Trace the Bass program (same NEFF runs SPMD on all 8 cores)."""
    import concourse.bass as bass
    import concourse.tile as tile
    from concourse import bacc, mybir
    from concourse.masks import make_identity

    f32 = mybir.dt.float32
    i32 = mybir.dt.int32
    mm_dt = mybir.dt.float32r if stream_f32r else f32

    nc = bacc.Bacc()

    # ---- DRAM I/O ----------------------------------------------------------
    # X inputs carry the streaming-matmul dtype (float32r == float32 bits;
    # only the PE interprets it as tf32) so the BIR verifier sees consistent
    # producer/consumer dtypes on the fp32r path.
    xK_f = nc.dram_tensor("xK_f", [BL, N, D], mm_dt, kind="ExternalInput")
    xk1_f = nc.dram_tensor("xk1_f", [BL, N, KD], mm_dt, kind="ExternalInput")
    xK_b = nc.dram_tensor("xK_b", [BL, N, D], mm_dt, kind="ExternalInput")
    xk1_b = nc.dram_tensor("xk1_b", [BL, N, KD], mm_dt, kind="ExternalInput")
    adj_f = nc.dram_tensor("adj_f", [BL, N], i32, kind="ExternalInput")
    sm_f = nc.dram_tensor("sm_f", [BL, N], i32, kind="ExternalInput")
    adj_b = nc.dram_tensor("adj_b", [BL, N], i32, kind="ExternalInput")
    sm_b = nc.dram_tensor("sm_b", [BL, N], i32, kind="ExternalInput")
    v_f = nc.dram_tensor("v_f", [F], f32, kind="ExternalInput")
    v_b = nc.dram_tensor("v_b", [F], f32, kind="ExternalInput")
    G0_f = nc.dram_tensor("G0_f", [F + 1, D], f32, kind="ExternalInput")
    G1_f = nc.dram_tensor("G1_f", [F + 1, D], f32, kind="ExternalInput")
    G0_b = nc.dram_tensor("G0_b", [F + 1, D], f32, kind="ExternalInput")
    G1_b = nc.dram_tensor("G1_b", [F + 1, D], f32, kind="ExternalInput")
    out_f = nc.dram_tensor("out_f", [BL, D], f32, kind="ExternalOutput")
    out_b = nc.dram_tensor("out_b", [BL, D], f32, kind="ExternalOutput")

    branches = [
        dict(xK=xK_f, xk1=xk1_f, adj=adj_f, sm=sm_f, v=v_f, G0=G0_f, G1=G1_f, out=out_f),
        dict(xK=xK_b, xk1=xk1_b, adj=adj_b, sm=sm_b, v=v_b, G0=G0_b, G1=G1_b, out=out_b),
    ]

    with tile.TileContext(nc) as tc:
        with (
            tc.tile_pool(name="singles", bufs=1) as singles,
            tc.tile_pool(name="xKp", bufs=2) as xKp,
            tc.tile_pool(name="xk1p", bufs=3) as xk1p,
            tc.tile_pool(name="scr", bufs=3) as scr,
            tc.tile_pool(name="small", bufs=4) as small,
            tc.tile_pool(name="uallp", bufs=2) as uallp,
            tc.tile_pool(name="uallTp", bufs=2) as uallTp,
            tc.tile_pool(name="finp", bufs=2) as finp,
            tc.tile_pool(name="psU_K", bufs=2, space="PSUM") as psU_K,
            tc.tile_pool(name="psU_1", bufs=2, space="PSUM") as psU_1,
            tc.tile_pool(name="psTr", bufs=2, space="PSUM") as psTr,
            tc.tile_pool(name="psOut", bufs=1, space="PSUM") as psOut,
        ):
            # ---- one-time setup -------------------------------------------
            ident = singles.tile([128, 128], f32)
            make_identity(nc, ident)
            ones11 = singles.tile([1, 1], f32)
            nc.vector.memset(ones11, 1.0)
            # f32 zeros used to produce f32r-typed zeros/ones (memset can't
            # write f32r, but tensor ops can)
            zf = singles.tile([128, NT, 8], f32)
            nc.vector.memset(zf, 0.0)
            # (128,2) ones in the matmul dtype: moving operand of the
            # softmax-denominator accumulation
            ones2 = singles.tile([128, 2], mm_dt)
            nc.vector.tensor_scalar_add(out=ones2, in0=zf[:, 0, 0:2], scalar1=1.0)

            per_br = []
            for br in branches:
                st = {}
                # score vector broadcast across partitions: (128, 768)
                vb = singles.tile([128, F], f32)
                vap = br["v"][:]
                nc.gpsimd.dma_start(
                    out=vb,
                    in_=bass.AP(tensor=vap.tensor, offset=vap.offset, ap=[[0, 128]] + vap.ap),
                )
                st["vb"] = vb
                # G matrices: (128, 7, 512); chunk 6 row 0 holds row 768
                for gname in ("G0", "G1"):
                    g = br[gname]
                    gs = singles.tile([128, 7, D], f32)
                    nc.gpsimd.dma_start(
                        out=gs[:, 0:6, :],
                        in_=g[0:F, :].rearrange("(k p) n -> p k n", p=128),
                    )
                    nc.gpsimd.dma_start(out=gs[0:1, 6, :], in_=g[F : F + 1, :])
                    st[gname] = gs
                # masks for all BL batches: (128, BL, NT), token = p*NT + n
                adj_i = small.tile([128, BL, NT], i32, tag="mask_i")
                sm_i = small.tile([128, BL, NT], i32, tag="mask_i")
                nc.gpsimd.dma_start(out=adj_i, in_=br["adj"].rearrange("b (p n) -> p b n", n=NT))
                nc.gpsimd.dma_start(out=sm_i, in_=br["sm"].rearrange("b (p n) -> p b n", n=NT))
                adjf = small.tile([128, BL, NT], f32, tag="mask_f")
                smf = small.tile([128, BL, NT], f32, tag="mask_f")
                nc.vector.tensor_copy(adjf, adj_i)
                nc.vector.tensor_copy(smf, sm_i)
                m0 = singles.tile([128, BL, NT], f32, tag=f"m0_{br['out'].name}")
                m1 = singles.tile([128, BL, NT], f32, tag=f"m1_{br['out'].name}")
                nc.vector.tensor_mul(m0, adjf, smf)
                nc.vector.tensor_sub(m1, adjf, m0)
                st["m0"], st["m1"] = m0, m1
                per_br.append(st)

            # ---- streaming + finishing per branch -------------------------
            for bi, br in enumerate(branches):
                st = per_br[bi]
                psK = psU_K.tile([8, D], f32)       # rows 0-3: U0(b), rows 4-7: U1(b)
                # cols 0:KD = U_k1, col KD = ones column (P0/P1), col KD+1 = pad
                # (fp32r matmul needs even moving-free-size / 8B alignment)
                ps1 = psU_1.tile([8, KD + 2], f32)

                for b in range(BL):
                    # contiguous-destination tiles keep SWDGE descriptor
                    # generation cheap (strided dst was costing ~13us/unit on Q7)
                    xK = xKp.tile([128, NT, D], mm_dt, tag="xK")
                    nc.gpsimd.dma_start(
                        out=xK, in_=br["xK"][b].rearrange("(p n) d -> p n d", n=NT)
                    )
                    xk1 = xk1p.tile([128, NT, KD], mm_dt, tag="xk1")
                    nc.gpsimd.dma_start(
                        out=xk1, in_=br["xk1"][b].rearrange("(p n) d -> p n d", n=NT)
                    )
                    xK_f32 = xK[:, :, :].bitcast(f32)
                    xk1_f32 = xk1[:, :, :].bitcast(f32)

                    sA = small.tile([128, NT], f32, tag="sA")
                    sB = small.tile([128, NT], f32, tag="sB")
                    prodK = scr.tile([128, D], f32, tag="prodK")
                    prod1 = scr.tile([128, KD], f32, tag="prod1")
                    for n in range(NT):
                        nc.vector.scalar_tensor_tensor(
                            out=prodK,
                            in0=xK_f32[:, n, :],
                            scalar=0.0,
                            in1=st["vb"][:, 0:D],
                            op0=mybir.AluOpType.bypass,
                            op1=mybir.AluOpType.mult,
                            accum_out=sA[:, n : n + 1],
                        )
                        nc.vector.scalar_tensor_tensor(
                            out=prod1,
                            in0=xk1_f32[:, n, :],
                            scalar=0.0,
                            in1=st["vb"][:, D:F],
                            op0=mybir.AluOpType.bypass,
                            op1=mybir.AluOpType.mult,
                            accum_out=sB[:, n : n + 1],
                        )
                    nc.vector.tensor_add(sB, sA, sB)
                    p_raw = small.tile([128, NT], f32, tag="p_raw")
                    nc.scalar.activation(out=p_raw, in_=sB, func=mybir.ActivationFunctionType.Exp)

                    # pp[:, n, :]: col b = p*adj*sm, col 4+b = p*adj*(1-sm), rest 0
                    pp = small.tile([128, NT, 8], mm_dt, tag="pp")
                    nc.vector.tensor_mul(pp, zf, zf)
                    nc.vector.tensor_mul(pp[:, :, b], p_raw, st["m0"][:, b, :])
                    nc.vector.tensor_mul(pp[:, :, 4 + b], p_raw, st["m1"][:, b, :])

                    for n in range(NT):
                        first = b == 0 and n == 0
                        last = b == BL - 1 and n == NT - 1
                        nc.tensor.matmul(psK, pp[:, n, :], xK[:, n, :], start=first, stop=last)
                        # k1 accumulate + softmax-denominator ones column share
                        # one PSUM group (partial-width writes accumulate fine)
                        nc.tensor.matmul(
                            ps1[:, 0:KD], pp[:, n, :], xk1[:, n, :], start=first, stop=False
                        )
                        nc.tensor.matmul(
                            ps1[:, KD : KD + 2],
                            pp[:, n, :],
                            ones2,
                            start=False,
                            stop=last,
                        )

                # ---- finishing: out = (U0@G0 + U1@G1) / P ------------------
                uall = uallp.tile([8, F + 1], f32)
                nc.vector.tensor_copy(uall[:, 0:D], psK)
                nc.vector.tensor_copy(uall[:, D : F + 1], ps1[:, 0 : KD + 1])

                uallT = uallTp.tile([128, 7, 8], f32)
                for k in range(6):
                    trp = psTr.tile([128, 8], f32)
                    nc.tensor.transpose(trp, uall[:, k * 128 : (k + 1) * 128], ident[0:8, 0:8])
                    nc.vector.tensor_copy(uallT[:, k, :], trp)
                trp = psTr.tile([128, 8], f32)
                nc.tensor.transpose(trp[0:1, :], uall[:, F : F + 1], ident[0:8, 0:8])
                nc.vector.tensor_copy(uallT[0:1, 6, :], trp[0:1, :])

                po = psOut.tile([4, D + 1], f32)  # cols 0:D main, col D = P (bank 2)
                for k in range(6):
                    nc.tensor.matmul(
                        po[:, 0:D], uallT[:, k, 0:4], st["G0"][:, k, :], start=(k == 0), stop=False
                    )
                nc.tensor.matmul(
                    po[:, 0:D], uallT[0:1, 6, 0:4], st["G0"][0:1, 6, :], start=False, stop=False
                )
                for k in range(6):
                    nc.tensor.matmul(
                        po[:, 0:D], uallT[:, k, 4:8], st["G1"][:, k, :], start=False, stop=False
                    )
                nc.tensor.matmul(
                    po[:, 0:D], uallT[0:1, 6, 4:8], st["G1"][0:1, 6, :], start=False, stop=True
                )
                nc.tensor.matmul(po[:, D : D + 1], uallT[0:1, 6, 0:4], ones11, start=True, stop=False)
                nc.tensor.matmul(po[:, D : D + 1], uallT[0:1, 6, 4:8], ones11, start=False, stop=True)

                rp = finp.tile([4, 1], f32, tag="rp")
                nc.vector.reciprocal(rp, po[:, D : D + 1])
                osb = finp.tile([4, D], f32, tag="osb")
                nc.vector.tensor_scalar_mul(out=osb, in0=po[:, 0:D], scalar1=rp)
                nc.sync.dma_start(out=br["out"][:, :], in_=osb)

    nc.compile()
    return nc


def _get_nc(stream_f32r: bool):
    key = ("nc", stream_f32r)
    if key not in _BUILD_CACHE:
        _BUILD_CACHE[key] = _build(stream_f32r)
    return _BUILD_CACHE[key]


def kernel(**inputs) -> tuple:
    global last_results
    from concourse.bass_utils import run_bass_kernel_spmd

    f32 = np.float32
    K = np.ascontiguousarray(np.asarray(inputs["K"], dtype=f32))
    front_k1 = np.ascontiguousarray(np.asarray(inputs["front_k1"], dtype=f32))
    back_K = np.ascontiguousarray(np.asarray(inputs["back_K"], dtype=f32))
    back_k2 = np.ascontiguousarray(np.asarray(inputs["back_k2"], dtype=f32))
    Wfk = np.asarray(inputs["Wfk"], dtype=f32)
    bfk = np.asarray(inputs["bfk"], dtype=f32)
    Wbk = np.asarray(inputs["Wbk"], dtype=f32)
    bbk = np.asarray(inputs["bbk"], dtype=f32)
    Wr0 = np.asarray(inputs["Wr0"], dtype=f32)
    Wr1 = np.asarray(inputs["Wr1"], dtype=f32)
    wf_den = np.asarray(inputs["wf_den"], dtype=f32)
    wb_den = np.asarray(inputs["wb_den"], dtype=f32)
    adj_f = np.ascontiguousarray(np.asarray(inputs["front_sdj_den"], dtype=np.int32))
    sm_f = np.ascontiguousarray(np.asarray(inputs["front_s_mask"], dtype=np.int32))
    adj_b = np.ascontiguousarray(np.asarray(inputs["back_sdj_den"], dtype=np.int32))
    sm_b = np.ascontiguousarray(np.asarray(inputs["back_s_mask"], dtype=np.int32))
    i = int(np.asarray(inputs["i"]))
    num_utter = int(np.asarray(inputs["num_utter"]))

    # host-folded weights
    v_f = (Wfk.astype(np.float64) @ wf_den[D:].astype(np.float64)).astype(f32)
    v_b = (Wbk.astype(np.float64) @ wb_den[D:].astype(np.float64)).astype(f32)
    A_f = np.vstack([Wfk, bfk[None, :]]).astype(np.float64)
    A_b = np.vstack([Wbk, bbk[None, :]]).astype(np.float64)
    G0_f = (A_f @ Wr0.astype(np.float64)).astype(f32)
    G1_f = (A_f @ Wr1.astype(np.float64)).astype(f32)
    G0_b = (A_b @ Wr0.astype(np.float64)).astype(f32)
    G1_b = (A_b @ Wr1.astype(np.float64)).astype(f32)

    stream_f32r = os.environ.get("KERNEL_MM_F32R", "1") == "1"
    nc = _get_nc(stream_f32r)

    in_maps = []
    for c in range(NCORES):
        s = slice(c * BL, (c + 1) * BL)
        in_maps.append(
            {
                "xK_f": K[s],
                "xk1_f": front_k1[s],
                "xK_b": back_K[s],
                "xk1_b": back_k2[s],
                "adj_f": adj_f[s],
                "sm_f": sm_f[s],
                "adj_b": adj_b[s],
                "sm_b": sm_b[s],
                "v_f": v_f,
                "v_b": v_b,
                "G0_f": G0_f,
                "G1_f": G1_f,
                "G0_b": G0_b,
                "G1_b": G1_b,
            }
        )

    trace = os.environ.get("KERNEL_TRACE", "0") == "1"
    res = run_bass_kernel_spmd(nc, in_maps, core_ids=list(range(NCORES)), trace=trace)
    last_results = res

    front = np.concatenate([r["out_f"] for r in res.results], axis=0)
    back = np.concatenate([r["out_b"] for r in res.results], axis=0)
    if i == 0:
        front = np.zeros((B, D), dtype=f32)
    if i == num_utter - 1:
        back = np.zeros((B, D), dtype=f32)
    return (front, back)



# revision 6
# speedup vs baseline: 1.3975x; 1.3975x over previous
"""Trainium2 Bass kernel for the DialogGCN GAT-style message-passing layer.

Math notes (why this is much cheaper than the reference graph):
  Kp    = concat(K, kfeat) @ Wk + bk                    (B,N,D)
  alpha = Q@wden[:D] + Kp@wden[D:] + bden               (B,N)
  w     = softmax(alpha - (1-adj)*1e30, axis=N)
  out   = sum_n w * ((Kp@Wr0)*sm + (Kp@Wr1)*(1-sm))

* softmax is invariant to per-row constants, so the Q term, bden and the
  bk@wden[D:] constant all cancel:  w = softmax_n(X_n . v) masked, where
  X = concat(K, kfeat) and v = Wk @ wden[D:]  (folded on host).
* the output is linear in the weighted sums:
    out = (sum_n w*sm*[X|1]) @ G0 + (sum_n w*(1-sm)*[X|1]) @ G1
  with G0 = [Wk;bk]@Wr0, G1 = [Wk;bk]@Wr1 folded on host (769x512 each).
  The device streams X once computing
    s_n = X_n . v ; p_n = exp(s_n) ; U0 = sum p*m0*[X|1] ; U1 = sum p*m1*[X|1]
  (m0 = adj*sm, m1 = adj*(1-sm)) followed by out = (U0@G0 + U1@G1) / P,
  where P = U0[768] + U1[768] (the ones-column sums).

All X traffic is fp16 (host-cast): halves HBM bytes and doubles both the
DVE score throughput (2x_1P packed mode) and PE matmul rate. K, k1 and the
ones columns are packed host-side into one [N, 772] array so each token
tile needs a single contiguous DMA, one 768-wide score dot-product on DVE,
and two accumulation matmuls on PE.

Sharding: pure data parallel over batch B=32 across 8 cores (4 rows each).
"""

import os
import sys

import numpy as np

for _p in ("/opt/trn_rl_repo", "/root/.axon_site/_ro/trn_rl_repo"):
    if os.path.isdir(_p) and _p not in sys.path:
        sys.path.insert(0, _p)

B, N, D, KD = 32, 2048, 512, 256
F = D + KD  # 768
FP = F + 4  # 772: cols 768:772 are ones (P accum); pad keeps 8B alignment
NCORES = 8
BL = B // NCORES  # 4 batch rows per core
NT = 16  # free-dim token tiles per batch (N = 128 * NT)
CH = 8  # n-tiles per score->exp->matmul chunk (keeps PE fed, short tail)

_BUILD_CACHE = {}
last_results = None  # BassKernelResults of the most recent run (for test.py)


def _build():
    """Trace the Bass program (same NEFF runs SPMD on all 8 cores)."""
    import concourse.bass as bass
    import concourse.tile as tile
    from concourse import bacc, mybir
    from concourse.masks import make_identity

    f32 = mybir.dt.float32
    f16 = mybir.dt.float16

    nc = bacc.Bacc()

    # ---- DRAM I/O ----------------------------------------------------------
    xm_f = nc.dram_tensor("xm_f", [BL, N, FP], f16, kind="ExternalInput")
    xm_b = nc.dram_tensor("xm_b", [BL, N, FP], f16, kind="ExternalInput")
    m0_f = nc.dram_tensor("m0_f", [128, BL, NT], f16, kind="ExternalInput")
    m1_f = nc.dram_tensor("m1_f", [128, BL, NT], f16, kind="ExternalInput")
    m0_b = nc.dram_tensor("m0_b", [128, BL, NT], f16, kind="ExternalInput")
    m1_b = nc.dram_tensor("m1_b", [128, BL, NT], f16, kind="ExternalInput")
    v_f = nc.dram_tensor("v_f", [128, F], f16, kind="ExternalInput")
    v_b = nc.dram_tensor("v_b", [128, F], f16, kind="ExternalInput")
    g0_f = nc.dram_tensor("g0_f", [128, 7, D], f16, kind="ExternalInput")
    g1_f = nc.dram_tensor("g1_f", [128, 7, D], f16, kind="ExternalInput")
    g0_b = nc.dram_tensor("g0_b", [128, 7, D], f16, kind="ExternalInput")
    g1_b = nc.dram_tensor("g1_b", [128, 7, D], f16, kind="ExternalInput")
    out_f = nc.dram_tensor("out_f", [BL, D], f32, kind="ExternalOutput")
    out_b = nc.dram_tensor("out_b", [BL, D], f32, kind="ExternalOutput")

    branches = [
        dict(xm=xm_f, m0=m0_f, m1=m1_f, v=v_f, g0=g0_f, g1=g1_f, out=out_f),
        dict(xm=xm_b, m0=m0_b, m1=m1_b, v=v_b, g0=g0_b, g1=g1_b, out=out_b),
    ]

    with tile.TileContext(nc) as tc:
        with (
            tc.tile_pool(name="singles", bufs=1) as singles,
            tc.tile_pool(name="xmp", bufs=4) as xmp,
            tc.tile_pool(name="scr", bufs=2) as scr,
            tc.tile_pool(name="small", bufs=3) as small,
            tc.tile_pool(name="uallp", bufs=2) as uallp,
            tc.tile_pool(name="uallTp", bufs=2) as uallTp,
            tc.tile_pool(name="finp", bufs=2) as finp,
            tc.tile_pool(name="psU_K", bufs=1, space="PSUM") as psU_K,
            tc.tile_pool(name="psU_1", bufs=1, space="PSUM") as psU_1,
            tc.tile_pool(name="psTr", bufs=2, space="PSUM") as psTr,
            tc.tile_pool(name="psOut", bufs=2, space="PSUM") as psOut,
        ):
            # ---- one-time setup -------------------------------------------
            ident = singles.tile([128, 128], f32)
            make_identity(nc, ident)

            st = []
            for br in branches:
                d = {}
                vb = singles.tile([128, F], f16, tag=f"vb_{br['v'].name}")
                nc.gpsimd.dma_start(out=vb, in_=br["v"][:, :])
                d["vb"] = vb
                for gn in ("g0", "g1"):
                    gs = singles.tile([128, 7, D], f16, tag=f"{gn}_{br['out'].name}")
                    nc.gpsimd.dma_start(out=gs, in_=br[gn][:, :, :])
                    d[gn] = gs
                for mn in ("m0", "m1"):
                    ms = singles.tile([128, BL, NT], f16, tag=f"{mn}_{br['out'].name}")
                    nc.gpsimd.dma_start(out=ms, in_=br[mn][:, :, :])
                    d[mn] = ms
                d["psK"] = psU_K.tile([8, D], f32, name=f"psK_{br['out'].name}")
                d["ps1"] = psU_1.tile([8, FP - D], f32, name=f"ps1_{br['out'].name}")
                st.append(d)

            # ---- streaming: interleave branches so finishing overlaps -----
            units = [(bi, b) for b in range(BL) for bi in range(2)]
            for bi, b in units:
                br, sd = branches[bi], st[bi]
                xm = xmp.tile([128, NT, FP], f16, tag="xm")
                nc.gpsimd.dma_start(
                    out=xm, in_=br["xm"][b].rearrange("(p n) d -> p n d", n=NT)
                )

                sB = small.tile([128, NT], f32, tag="sB")
                pp = small.tile([128, 8, NT], f16, tag="pp")
                nc.vector.memset(pp, 0.0)
                prod = scr.tile([128, F], f16, tag="prod")

                for c in range(NT // CH):
                    lo, hi = c * CH, (c + 1) * CH
                    for n in range(lo, hi):
                        nc.vector.scalar_tensor_tensor(
                            out=prod,
                            in0=xm[:, n, 0:F],
                            scalar=0.0,
                            in1=sd["vb"],
                            op0=mybir.AluOpType.bypass,
                            op1=mybir.AluOpType.mult,
                            accum_out=sB[:, n : n + 1],
                        )
                    pr = scr.tile([128, CH], f16, tag="pr")
                    nc.scalar.activation(
                        out=pr, in_=sB[:, lo:hi], func=mybir.ActivationFunctionType.Exp
                    )
                    nc.vector.tensor_mul(pp[:, b, lo:hi], pr, sd["m0"][:, b, lo:hi])
                    nc.vector.tensor_mul(pp[:, 4 + b, lo:hi], pr, sd["m1"][:, b, lo:hi])
                    for n in range(lo, hi):
                        first = b == 0 and n == 0
                        last = b == BL - 1 and n == NT - 1
                        nc.tensor.matmul(
                            sd["psK"], pp[:, :, n], xm[:, n, 0:D], start=first, stop=last
                        )
                        nc.tensor.matmul(
                            sd["ps1"], pp[:, :, n], xm[:, n, D:FP], start=first, stop=last
                        )

                # ---- finishing: out = (U0@G0 + U1@G1) / P ------------------
                if b == BL - 1:
                    uall = uallp.tile([8, F + 1], f32)
                    nc.vector.tensor_copy(uall[:, 0:D], sd["psK"])
                    nc.vector.tensor_copy(uall[:, D : F + 1], sd["ps1"][:, 0 : KD + 1])

                    uallT = uallTp.tile([128, 7, 8], f16)
                    for k in range(6):
                        trp = psTr.tile([128, 8], f32, tag="trp")
                        nc.tensor.transpose(
                            trp, uall[:, k * 128 : (k + 1) * 128], ident[0:8, 0:8]
                        )
                        nc.vector.tensor_copy(uallT[:, k, :], trp)
                    trp = psTr.tile([128, 8], f32, tag="trp")
                    nc.tensor.transpose(trp[0:1, :], uall[:, F : F + 1], ident[0:8, 0:8])
                    nc.vector.tensor_copy(uallT[0:1, 6, :], trp[0:1, :])

                    # P(b) = U0_768(b) + U1_768(b); rp = 1/P transposed to [4,1]
                    prow = finp.tile([1, 8], f32, tag="prow")
                    nc.vector.tensor_copy(prow, trp[0:1, :])
                    padd = finp.tile([1, 4], f32, tag="padd")
                    nc.vector.tensor_add(padd, prow[0:1, 0:4], prow[0:1, 4:8])
                    rrow = finp.tile([1, 4], f32, tag="rrow")
                    nc.vector.reciprocal(rrow, padd)
                    trp2 = psTr.tile([4, 1], f32, tag="trp")
                    nc.tensor.transpose(trp2, rrow, ident[0:1, 0:1])
                    rp = finp.tile([4, 1], f32, tag="rp")
                    nc.vector.tensor_copy(rp, trp2)

                    po = psOut.tile([4, D], f32)
                    for k in range(6):
                        nc.tensor.matmul(
                            po, uallT[:, k, 0:4], sd["g0"][:, k, :],
                            start=(k == 0), stop=False,
                        )
                    nc.tensor.matmul(
                        po, uallT[0:1, 6, 0:4], sd["g0"][0:1, 6, :],
                        start=False, stop=False,
                    )
                    for k in range(6):
                        nc.tensor.matmul(
                            po, uallT[:, k, 4:8], sd["g1"][:, k, :],
                            start=False, stop=False,
                        )
                    nc.tensor.matmul(
                        po, uallT[0:1, 6, 4:8], sd["g1"][0:1, 6, :],
                        start=False, stop=True,
                    )

                    osb = finp.tile([4, D], f32, tag="osb")
                    nc.vector.tensor_scalar_mul(out=osb, in0=po, scalar1=rp)
                    nc.sync.dma_start(out=br["out"][:, :], in_=osb)

    nc.compile()
    return nc


def _get_nc():
    if "nc" not in _BUILD_CACHE:
        _BUILD_CACHE["nc"] = _build()
    return _BUILD_CACHE["nc"]


def _pack_x(K, k1):
    """[B,N,772] fp16 = [K | k1 | 1,1,1,1] (ones columns accumulate P)."""
    out = np.empty((B, N, FP), np.float16)
    out[..., :D] = K
    out[..., D:F] = k1
    out[..., F:] = 1.0
    return out


def _pack_g(A64, Wr):
    """[Wk;bk]@Wr folded to the on-chip [128, 7, D] chunk layout, fp16."""
    G = (A64 @ Wr.astype(np.float64)).astype(np.float16)  # (769, 512)
    out = np.zeros((128, 7, D), np.float16)
    out[:, 0:6, :] = G[0:F].reshape(6, 128, D).transpose(1, 0, 2)
    out[0, 6, :] = G[F]
    return out


def _pack_mask(m, c):
    """(B,N) 0/1 float -> per-core [128, BL, NT] fp16 (token = p*NT + n)."""
    mc = m[c * BL : (c + 1) * BL]
    return np.ascontiguousarray(
        mc.reshape(BL, 128, NT).transpose(1, 0, 2).astype(np.float16)
    )


def kernel(**inputs) -> tuple:
    global last_results
    from concourse.bass_utils import run_bass_kernel_spmd

    f32 = np.float32
    K = np.asarray(inputs["K"], dtype=f32)
    front_k1 = np.asarray(inputs["front_k1"], dtype=f32)
    back_K = np.asarray(inputs["back_K"], dtype=f32)
    back_k2 = np.asarray(inputs["back_k2"], dtype=f32)
    Wfk = np.asarray(inputs["Wfk"], dtype=f32)
    bfk = np.asarray(inputs["bfk"], dtype=f32)
    Wbk = np.asarray(inputs["Wbk"], dtype=f32)
    bbk = np.asarray(inputs["bbk"], dtype=f32)
    Wr0 = np.asarray(inputs["Wr0"], dtype=f32)
    Wr1 = np.asarray(inputs["Wr1"], dtype=f32)
    wf_den = np.asarray(inputs["wf_den"], dtype=f32)
    wb_den = np.asarray(inputs["wb_den"], dtype=f32)
    adj_f = np.asarray(inputs["front_sdj_den"], dtype=f32)
    sm_f = np.asarray(inputs["front_s_mask"], dtype=f32)
    adj_b = np.asarray(inputs["back_sdj_den"], dtype=f32)
    sm_b = np.asarray(inputs["back_s_mask"], dtype=f32)
    i = int(np.asarray(inputs["i"]))
    num_utter = int(np.asarray(inputs["num_utter"]))

    # host-folded weights (parameter preprocessing only)
    v_f = np.broadcast_to(
        (Wfk.astype(np.float64) @ wf_den[D:].astype(np.float64)).astype(np.float16),
        (128, F),
    )
    v_b = np.broadcast_to(
        (Wbk.astype(np.float64) @ wb_den[D:].astype(np.float64)).astype(np.float16),
        (128, F),
    )
    v_f = np.ascontiguousarray(v_f)
    v_b = np.ascontiguousarray(v_b)
    A_f = np.vstack([Wfk, bfk[None, :]]).astype(np.float64)
    A_b = np.vstack([Wbk, bbk[None, :]]).astype(np.float64)
    G0_f = _pack_g(A_f, Wr0)
    G1_f = _pack_g(A_f, Wr1)
    G0_b = _pack_g(A_b, Wr0)
    G1_b = _pack_g(A_b, Wr1)

    # input marshaling: fp16 cast + layout packing
    X_f = _pack_x(K, front_k1)
    X_b = _pack_x(back_K, back_k2)
    m0f, m1f = adj_f * sm_f, adj_f * (1.0 - sm_f)
    m0b, m1b = adj_b * sm_b, adj_b * (1.0 - sm_b)

    nc = _get_nc()

    in_maps = []
    for c in range(NCORES):
        s = slice(c * BL, (c + 1) * BL)
        in_maps.append(
            {
                "xm_f": X_f[s],
                "xm_b": X_b[s],
                "m0_f": _pack_mask(m0f, c),
                "m1_f": _pack_mask(m1f, c),
                "m0_b": _pack_mask(m0b, c),
                "m1_b": _pack_mask(m1b, c),
                "v_f": v_f,
                "v_b": v_b,
                "g0_f": G0_f,
                "g1_f": G1_f,
                "g0_b": G0_b,
                "g1_b": G1_b,
            }
        )

    trace = os.environ.get("KERNEL_TRACE", "0") == "1"
    res = run_bass_kernel_spmd(nc, in_maps, core_ids=list(range(NCORES)), trace=trace)
    last_results = res

    front = np.concatenate([r["out_f"] for r in res.results], axis=0)
    back = np.concatenate([r["out_b"] for r in res.results], axis=0)
    if i == 0:
        front = np.zeros((B, D), dtype=f32)
    if i == num_utter - 1:
        back = np.zeros((B, D), dtype=f32)
    return (front, back)
